# revision 25
# baseline (speedup 1.0000x reference)
"""Trainium2 Bass kernel for nn_EquivariantConvolution (gnn_message_passing).

Math (per edge e):
    h  = relu(edge_feats @ W1 + b1)            [E,128]
    rw = (h @ W2 + b2) -> [E, 16, 48]
    fe = f[U]                                  [E,16,3]
    tmp[e,m,k] = sum_d fe[e,m,d] * basis[e,d,k]        (k = r*3+dd, 9)
    out[e,i,dd] = sum_{m,r} rw[e,i,m*3+r] * tmp[e,m,r*3+dd]

Sharding: edges split across 8 cores (40000 each, padded to 40960);
f + MLP weights replicated. Device layout: edge j of a 128-edge tile on
SBUF partition j%128.

fp16 pipeline: MLP on PE in fp16 (1 cyc/row), b2 bias via ones-matmul on
PE, rw evicted PSUM->SBUF as fp16 by the ACT engine, per-edge
contractions on DVE in fp16 with packed innermost dims (2x mode):
  step4 (tmp): products [p, (dd,r), m, d] (d innermost, packed in both
    operands via host layouts fe=[m,d], basis=[(dd,r),d]) + 2 adds.
  step5 (out): W2 cols host-reordered to (i,r,m) so rw [p,i,(r,m)] and
    tmp [p,dd,(r,m)] share a packed 48-wide innermost dim; products
    [p,i,dd,48] at 2x then halving adds 48->24->12->6->3 + reduce X.
Output fp16, host upcasts.
"""
import sys

sys.path.insert(0, "/opt/trn_rl_repo")

import os
import numpy as np
import concourse.bass as bass
import concourse.bacc as bacc
import concourse.mybir as mybir
import concourse.tile as tile
from concourse.bass_utils import run_bass_kernel_spmd
from contextlib import ExitStack

# problem constants (hardcoded per harness contract)
E = 320000
N = 10000
M1 = 16
M2 = 16
D1 = 3
D2 = 3
NREPS = 3
EDGE_DIM = 32
HIDDEN = 128
RW = NREPS * M1 * M2  # 768

NCORES = 8
ES = E // NCORES          # 40000 edges per core
ESP = 40960               # padded to 320 tiles of 128
NTILES = ESP // 128       # 320
BLK = int(os.environ.get('KBLK', '32'))        # tiles per block
NBLK = NTILES // BLK      # 20
EBLK = BLK * 128          # 2048 edges per block

_CACHE = {}

ABL = set(os.environ.get("KABL", "").split(","))  # ablation flags for benching
POOL4 = os.environ.get("KPOOL4", "0") == "1"      # step4 on gpsimd (slow: Q7)
HOSTG = os.environ.get("KHOSTG", "0") == "1"      # gather f[U] on host
FPAD = 128                # f rows padded to 128 fp16 (256B) for dma_gather


def _build(reps=1):
    dt = mybir.dt
    nc = bacc.Bacc("TRN2", target_bir_lowering=False, debug=False,
                   num_devices=NCORES)

    # DRAM tensors (per-core shards fed via in_maps)
    efT_d = nc.dram_tensor("efT", [EDGE_DIM, ESP], dt.float16, kind="ExternalInput").ap()
    basis_d = nc.dram_tensor("basisp", [128, NTILES * 27], dt.float16, kind="ExternalInput").ap()
    if HOSTG:
        fep_d = nc.dram_tensor("fep", [128, NTILES * 48], dt.float16, kind="ExternalInput").ap()
    else:
        uw_d = nc.dram_tensor("uw", [128, ESP // 16], dt.int16, kind="ExternalInput").ap()
        fpad_d = nc.dram_tensor("fpad", [N, FPAD], dt.float16, kind="ExternalInput").ap()
        fep_d = None
    w1_d = nc.dram_tensor("w1", [EDGE_DIM, HIDDEN], dt.float16, kind="ExternalInput").ap()
    b1_d = nc.dram_tensor("b1", [HIDDEN, 1], dt.float32, kind="ExternalInput").ap()
    w2_d = nc.dram_tensor("w2r", [HIDDEN, RW], dt.float16, kind="ExternalInput").ap()
    b2_d = nc.dram_tensor("b2r", [1, RW], dt.float16, kind="ExternalInput").ap()
    ones_d = nc.dram_tensor("ones1", [1, 128], dt.float16, kind="ExternalInput").ap()
    out_d = nc.dram_tensor("outp", [128, NTILES * 48], dt.float16, kind="ExternalOutput").ap()

    with tile.TileContext(nc) as tc, ExitStack() as ctx:
        cpool = ctx.enter_context(tc.tile_pool(name="const", bufs=1))
        inpool = ctx.enter_context(tc.tile_pool(name="in", bufs=4))
        hpool = ctx.enter_context(tc.tile_pool(name="h", bufs=2))
        wpool = ctx.enter_context(tc.tile_pool(name="work", bufs=6))
        opool = ctx.enter_context(tc.tile_pool(name="out", bufs=2))
        pps = ctx.enter_context(tc.tile_pool(name="psA", bufs=2, space="PSUM"))
        ppr = ctx.enter_context(tc.tile_pool(name="psB", bufs=3, space="PSUM"))

        # constants
        w1_sb = cpool.tile([EDGE_DIM, HIDDEN], dt.float16)
        nc.sync.dma_start(w1_sb[:], w1_d[:])
        b1_sb = cpool.tile([HIDDEN, 1], dt.float32)
        nc.sync.dma_start(b1_sb[:], b1_d[:])
        w2_sb = cpool.tile([HIDDEN, RW], dt.float16)
        nc.sync.dma_start(w2_sb[:], w2_d[:])
        b2_sb = cpool.tile([1, RW], dt.float16)
        nc.sync.dma_start(b2_sb[:], b2_d[:])
        ones_sb = cpool.tile([1, 128], dt.float16)
        nc.sync.dma_start(ones_sb[:], ones_d[:])
        if HOSTG:
            uw_sb = fpad_dd = None
        else:
            uw_sb = cpool.tile([128, ESP // 16], dt.int16)
            nc.sync.dma_start(uw_sb[:], uw_d[:])
            fpad_dd = fpad_d

        def body():
            _body(nc, tc, dt, cpool, inpool, hpool, wpool, opool, pps, ppr,
                  efT_d, basis_d, fep_d, uw_sb, fpad_dd,
                  w1_sb, b1_sb, w2_sb, b2_sb, ones_sb, out_d)

        if reps == 1:
            body()
        else:
            with tc.For_i(0, reps, 1):
                body()

    nc.compile()
    return nc


def _body(nc, tc, dt, cpool, inpool, hpool, wpool, opool, pps, ppr,
          efT_d, basis_d, fep_d, uw_sb, fpad_d,
          w1_sb, b1_sb, w2_sb, b2_sb, ones_sb, out_d):
    K9 = NREPS * D2   # 9
    RM = NREPS * M1   # 48
    FW = 48 if HOSTG else FPAD
    with nc.allow_low_precision(reason="fp16 pipeline; abs gate 2e-2"):
        for b in range(NBLK):
            # block loads
            efT_sb = inpool.tile([EDGE_DIM, EBLK], dt.float16, tag="efT")
            nc.sync.dma_start(efT_sb[:], efT_d[:, b * EBLK:(b + 1) * EBLK])
            basis_sb = inpool.tile([128, BLK * 27], dt.float16, tag="basis")
            nc.sync.dma_start(basis_sb[:], basis_d[:, b * BLK * 27:(b + 1) * BLK * 27])
            fe_sb = inpool.tile([128, BLK, FW], dt.float16, tag="fe")
            if HOSTG:
                nc.sync.dma_start(
                    fe_sb[:].rearrange("p b k -> p (b k)"),
                    fep_d[:, b * BLK * 48:(b + 1) * BLK * 48])
            else:
                for g in range(EBLK // 1024):
                    i0 = b * (EBLK // 16) + g * 64
                    nc.gpsimd.dma_gather(
                        fe_sb[:, g * 8:(g + 1) * 8, :], fpad_d[:],
                        uw_sb[:, i0:i0 + 64],
                        num_idxs=1024, num_idxs_reg=1024, elem_size=FPAD,
                    )

            # h.T = relu(W1.T @ efT + b1): [128h, EBLK] fp16
            hT_sb = hpool.tile([HIDDEN, EBLK], dt.float16, tag="hT")
            for q in range(EBLK // 512 if "nomlp" not in ABL else 0):
                hT_ps = pps.tile([HIDDEN, 512], dt.float32, tag="hTps")
                nc.tensor.matmul(hT_ps[:], w1_sb[:],
                                 efT_sb[:, q * 512:(q + 1) * 512],
                                 start=True, stop=True)
                nc.scalar.activation(hT_sb[:, q * 512:(q + 1) * 512], hT_ps[:],
                                     mybir.ActivationFunctionType.Relu,
                                     bias=b1_sb[:], scale=1.0)

            out_sb = opool.tile([128, BLK, 48], dt.float16, tag="outsb")

            for t in range(BLK):
                # rw = hT_chunk.T @ W2r + b2r : [128e, 768] in PSUM
                rw_ps = ppr.tile([128, RW], dt.float32, tag="rwps")
                hT_c = hT_sb[:, t * 128:(t + 1) * 128]
                if "nomlp" not in ABL:
                    nc.tensor.matmul(rw_ps[:, 0:512], hT_c, w2_sb[:, 0:512],
                                     start=True, stop=False)
                    nc.tensor.matmul(rw_ps[:, 0:512], ones_sb[:], b2_sb[:, 0:512],
                                     start=False, stop=True)
                    nc.tensor.matmul(rw_ps[:, 512:RW], hT_c, w2_sb[:, 512:RW],
                                     start=True, stop=False)
                    nc.tensor.matmul(rw_ps[:, 512:RW], ones_sb[:], b2_sb[:, 512:RW],
                                     start=False, stop=True)
                # ACT evicts + casts; frees the PSUM bank early
                rw_sb = wpool.tile([128, RW], dt.float16, tag="rwsb")
                nc.scalar.activation(rw_sb[:], rw_ps[:],
                                     mybir.ActivationFunctionType.Copy,
                                     bias=0.0, scale=1.0)

                # step4: tmp[p, (dd,r), m] = sum_d fe[p,m,d] * basis[p,(dd,r),d]
                # products with d innermost (packed in both operands -> 2x)
                eng4 = nc.gpsimd if POOL4 else nc.vector
                fe_t = fe_sb[:, t, 0:48].rearrange("p (m d) -> p m d", m=M1, d=D1)
                ba_t = basis_sb[:, t * 27:(t + 1) * 27].rearrange(
                    "p (k d) -> p k d", k=K9, d=D1)
                p4 = wpool.tile([128, K9, M1, D1], dt.float16, tag="p4")
                if "notmp" not in ABL:
                    fe_b = fe_t.unsqueeze(1).broadcast_to([128, K9, M1, D1])
                    ba_b = ba_t.unsqueeze(2).broadcast_to([128, K9, M1, D1])
                    eng4.tensor_tensor(p4[:], fe_b, ba_b, mybir.AluOpType.mult)
                tmp_sb = wpool.tile([128, K9, M1], dt.float16, tag="tmp")
                t01 = wpool.tile([128, K9, M1], dt.float16, tag="t01")
                if "notmp" not in ABL:
                    eng4.tensor_tensor(t01[:], p4[:, :, :, 0], p4[:, :, :, 1],
                                       mybir.AluOpType.add)
                    eng4.tensor_tensor(tmp_sb[:], t01[:], p4[:, :, :, 2],
                                       mybir.AluOpType.add)

                # step5: out[p,i,dd] = sum_{rm} rw[p,i,rm] * tmp[p,dd,rm]
                if "nostep5" not in ABL:
                    rw_b = rw_sb[:].rearrange(
                        "p (i rm) -> p i rm", i=M2, rm=RM
                    ).unsqueeze(2).broadcast_to([128, M2, D2, RM])
                    tmp_b = tmp_sb[:].rearrange(
                        "p (dd r) m -> p dd (r m)", dd=D2, r=NREPS
                    ).unsqueeze(1).broadcast_to([128, M2, D2, RM])
                    # products in two pre-split halves; their add IS the
                    # first tree level (one fewer pass over 2304 elems)
                    p5a = wpool.tile([128, M2, D2, 24], dt.float16, tag="p5a")
                    nc.vector.tensor_tensor(p5a[:], rw_b[:, :, :, 0:24],
                                            tmp_b[:, :, :, 0:24],
                                            mybir.AluOpType.mult)
                    p5b = wpool.tile([128, M2, D2, 24], dt.float16, tag="p5b")
                    nc.vector.tensor_tensor(p5b[:], rw_b[:, :, :, 24:48],
                                            tmp_b[:, :, :, 24:48],
                                            mybir.AluOpType.mult)
                    s24 = wpool.tile([128, M2, D2, 24], dt.float16, tag="s24")
                    nc.vector.tensor_tensor(s24[:], p5a[:], p5b[:],
                                            mybir.AluOpType.add)
                    s12 = wpool.tile([128, M2, D2, 12], dt.float16, tag="s12")
                    nc.vector.tensor_tensor(s12[:], s24[:, :, :, 0:12],
                                            s24[:, :, :, 12:24],
                                            mybir.AluOpType.add)
                    s6 = wpool.tile([128, M2, D2, 6], dt.float16, tag="s6")
                    nc.vector.tensor_tensor(s6[:], s12[:, :, :, 0:6],
                                            s12[:, :, :, 6:12],
                                            mybir.AluOpType.add)
                    s3 = wpool.tile([128, M2, D2, 3], dt.float16, tag="s3")
                    nc.vector.tensor_tensor(s3[:], s6[:, :, :, 0:3],
                                            s6[:, :, :, 3:6],
                                            mybir.AluOpType.add)
                    nc.vector.tensor_reduce(out_sb[:, t, :], s3[:],
                                            axis=mybir.AxisListType.X,
                                            op=mybir.AluOpType.add)

            if "nostep5" not in ABL:
                nc.sync.dma_start(out_d[:, b * BLK * 48:(b + 1) * BLK * 48],
                                  out_sb[:].rearrange("p b k -> p (b k)"))


def _get_nc(reps=1):
    key = ("nc", reps)
    if key not in _CACHE:
        _CACHE[key] = _build(reps)
    return _CACHE[key]


def _prep_core(U_c, basis_c, ef_c, f, W1, b1, W2, b2):
    """Build one core's input map (host-side layout/swizzle)."""
    npad = ESP - U_c.shape[0]
    U_p = np.concatenate([np.asarray(U_c, np.int64), np.zeros(npad, np.int64)])
    basis_p = np.concatenate(
        [np.asarray(basis_c, np.float32).reshape(-1, D1, NREPS * D2),
         np.zeros((npad, D1, NREPS * D2), np.float32)], axis=0)
    ef_p = np.concatenate(
        [np.asarray(ef_c, np.float32),
         np.zeros((npad, EDGE_DIM), np.float32)], axis=0)

    efT = np.ascontiguousarray(ef_p.T).astype(np.float16)                # [32, ESP]
    # basis inner layout per edge: (dd, r, d): idx = dd*9 + r*3 + d from [d, r*3+dd]
    bp = basis_p.reshape(ESP, D1, NREPS, D2)          # [e, d, r, dd]
    bp = bp.transpose(0, 3, 2, 1)                     # [e, dd, r, d]
    bp = bp.reshape(ESP, 27)
    basisp = np.ascontiguousarray(
        bp.reshape(NTILES, 128, 27).transpose(1, 0, 2).reshape(128, NTILES * 27)
    ).astype(np.float16)
    # source-node features: host-gathered stream, or gather table + indices
    f48 = np.asarray(f, np.float32).reshape(N, M1 * D1).astype(np.float16)
    if HOSTG:
        fe_all = f48[U_p]                                                # [ESP, 48]
        fmaps = {"fep": np.ascontiguousarray(
            fe_all.reshape(NTILES, 128, 48).transpose(1, 0, 2)
            .reshape(128, NTILES * 48))}
    else:
        uw16 = U_p.astype(np.int16).reshape(ESP // 1024, 64, 16).transpose(2, 0, 1)
        fpad = np.zeros((N, FPAD), np.float16)
        fpad[:, :M1 * D1] = f48
        fmaps = {
            "uw": np.ascontiguousarray(
                np.tile(uw16.reshape(16, ESP // 16), (8, 1))),
            "fpad": fpad,
        }
    # W2/b2 column reorder: (i, m, r) -> (i, r, m)
    w2r = np.asarray(W2, np.float32).reshape(HIDDEN, M2, M1, NREPS)
    w2r = np.ascontiguousarray(w2r.transpose(0, 1, 3, 2).reshape(HIDDEN, RW))
    b2r = np.asarray(b2, np.float32).reshape(M2, M1, NREPS)
    b2r = np.ascontiguousarray(b2r.transpose(0, 2, 1).reshape(1, RW))
    return {
        "efT": efT,
        "basisp": basisp,
        **fmaps,
        "w1": np.asarray(W1, np.float32).astype(np.float16),
        "b1": np.asarray(b1, np.float32).reshape(HIDDEN, 1),
        "w2r": w2r.astype(np.float16),
        "b2r": b2r.astype(np.float16),
        "ones1": np.ones((1, 128), np.float16),
    }


def kernel(U, basis, edge_feats, f, W1, b1, W2, b2):
    U = np.asarray(U)
    basis = np.asarray(basis, np.float32)
    edge_feats = np.asarray(edge_feats, np.float32)
    nc = _get_nc()
    in_maps = []
    for c in range(NCORES):
        sl = slice(c * ES, (c + 1) * ES)
        in_maps.append(_prep_core(U[sl], basis[sl], edge_feats[sl],
                                  f, W1, b1, W2, b2))
    res = run_bass_kernel_spmd(nc, in_maps, core_ids=list(range(NCORES)))
    outs = []
    for c in range(NCORES):
        op = res.results[c]["outp"]                                   # [128, NTILES*48] fp16
        o = op.astype(np.float32).reshape(128, NTILES, 48)
        o = o.transpose(1, 0, 2).reshape(ESP, 48)
        outs.append(o[:ES])
    return np.concatenate(outs, axis=0).reshape(E, M2, D2).astype(np.float32)


if __name__ == "__main__":
    # quick self-run with random data
    rng = np.random.default_rng(0)
    inputs = {
        "U": rng.integers(0, N, size=E),
        "basis": rng.standard_normal((E, D1, NREPS * D2), dtype=np.float32),
        "edge_feats": rng.standard_normal((E, EDGE_DIM), dtype=np.float32),
        "f": rng.standard_normal((N, M1, D1), dtype=np.float32),
        "W1": (rng.standard_normal((EDGE_DIM, HIDDEN), dtype=np.float32) / np.sqrt(EDGE_DIM)),
        "b1": rng.standard_normal(HIDDEN, dtype=np.float32) * 0.02,
        "W2": (rng.standard_normal((HIDDEN, RW), dtype=np.float32) / np.sqrt(HIDDEN)),
        "b2": rng.standard_normal(RW, dtype=np.float32) * 0.02,
    }
    out = kernel(**inputs)
    print(out.shape, out.dtype)
